# revision 5
# baseline (speedup 1.0000x reference)
"""Single-head attention on 8 Trainium2 NeuronCores.

Problem: x[8, 2048, 768], Wq/Wk/Wv[768, 64]+biases, mask[2048, 2048] int32
Output:  softmax(mask(Q K^T / 8)) V   -> [8, 2048, 64] f32

Sharding: data-parallel over batch — core b computes batch element b.

Per-core dataflow (all matmuls bf16 in / fp32 psum accumulate):
  host:  xT = x[b].T (w-major, partition-major relayout), Wqk = [Wq | Wk/8],
         mT = mask.T (k-major, 0/1 bf16, partition-major relayout)
  QK:    QK[n,128] = xT.T @ Wqk + bqk (bias via K=1 matmul), cast bf16 ->
         QQ/KK duplicated across partition halves (row-tiled score matmuls)
  V:     V[n,64] = xT.T @ Wv + bv, stored as V'[k,65] with ones column so the
         PV matmul also produces the softmax denominator for free
  ST:    ST[k,q] = KK.T @ QQ per 128-k-chunk (K=64 contraction: two chunks run
         concurrently in the PE array via row tiling at partitions 0/64)
  P:     P = exp(ST) on ScalarE (psum -> sbuf bf16), P *= mT (VectorE)
  OT:    OT[65,q] += V'[kchunk].T @ P[kchunk]  (accumulate over 16 k-chunks)
  out:   OT[65,2048] DMA'd straight from PSUM; host does the final
         out[q,h] = OT[h,q]/OT[64,q] normalization + transpose (gather step).
"""

import numpy as np
import ml_dtypes

import bass_rust
import concourse.bass as bass
import concourse.mybir as mybir
import concourse.tile as tile
from concourse.bass_utils import run_bass_kernel_spmd

BF16 = ml_dtypes.bfloat16
F32 = mybir.dt.float32
BF = mybir.dt.bfloat16

N_CORES = 8
SEQ = 2048
WIDTH = 768
HEAD = 64
NCH = WIDTH // 128      # 6 contraction chunks for the projections
NKC = SEQ // 128        # 16 key chunks
QT = 1024               # q tile (columns processed per main-loop sweep)
NQT = SEQ // QT


def _split_excess_waits(nc, max_waits=1):
    """walrus in this container rejects >1 sync wait per instruction; hoist
    extras onto preceding same-engine NoOps (same semantics: the engine
    executes its stream in order, so waiting earlier is equivalent)."""
    n = 0
    for bb in nc.main_func.blocks:
        new_list = []
        for ins in bb.instructions:
            si = ins.sync_info
            if si is not None and len(si.on_wait) > max_waits:
                waits = list(si.on_wait)
                extra, keep = waits[:-max_waits], waits[-max_waits:]
                for j, w in enumerate(extra):
                    nop = bass_rust.InstNoOp(
                        name=f"{ins.name}-ws{j}", engine=ins.engine, ins=[], outs=[]
                    )
                    nop.sync_info = mybir.SyncInfo(on_wait=[w], on_update=[])
                    new_list.append(nop)
                    n += 1
                ins.sync_info = mybir.SyncInfo(
                    on_wait=keep, on_update=list(si.on_update)
                )
            new_list.append(ins)
        bb.instructions = new_list
    return n


def _build():
    nc = bass.Bass("TRN2", target_bir_lowering=False, debug=False,
                   num_devices=N_CORES)

    # partition-major host layouts: row p holds everything partition p needs,
    # so each DMA is 128 large contiguous descriptors.
    xT_d = nc.declare_dram_parameter("xT", [128, 4 * NCH * 512], BF, False).ap()
    wqk_d = nc.declare_dram_parameter("Wqk", [128, NCH * 128], BF, False).ap()
    wv_d = nc.declare_dram_parameter("Wv", [128, NCH * HEAD], BF, False).ap()
    bqk_d = nc.declare_dram_parameter("bqk", [1, 128], BF, False).ap()
    bv_d = nc.declare_dram_parameter("bv", [1, HEAD], BF, False).ap()
    mT_d = nc.declare_dram_parameter("mT", [128, NKC * SEQ], BF, False).ap()
    ot_d = nc.declare_dram_parameter("ot", [HEAD + 1, SEQ], F32, True).ap()

    EXP = mybir.ActivationFunctionType.Exp

    with tile.TileContext(nc) as tc:
        with (
            tc.tile_pool(name="const", bufs=1) as const,
            tc.tile_pool(name="pp", bufs=4) as ppool,
            tc.tile_pool(name="ep", bufs=2) as epool,
            tc.tile_pool(name="stp", bufs=2, space="PSUM") as stp,
            tc.tile_pool(name="otp", bufs=1, space="PSUM") as otp,
            tc.tile_pool(name="scp", bufs=2, space="PSUM") as scp,
        ):
            # ---- inputs into SBUF (xT/W on the SP hardware-DGE path) ----
            xt = const.tile([128, 4, NCH, 512], BF)
            for t in range(4):
                nc.sync.dma_start(
                    out=xt[:, t, :, :],
                    in_=xT_d[:, t * NCH * 512:(t + 1) * NCH * 512],
                )
            wqk = const.tile([128, NCH, 128], BF)
            nc.sync.dma_start(out=wqk, in_=wqk_d)
            wv = const.tile([128, NCH, HEAD], BF)
            nc.sync.dma_start(out=wv, in_=wv_d)
            bqk = const.tile([1, 128], BF)
            nc.sync.dma_start(out=bqk, in_=bqk_d)
            bv = const.tile([1, HEAD], BF)
            nc.sync.dma_start(out=bv, in_=bv_d)
            ones = const.tile([1, 512], BF)
            nc.vector.memset(ones, 1.0)

            # mask streams on the software-DGE path (idle GpSimd) so it does
            # not sit in front of xT on the SP descriptor generator.
            mt = const.tile([128, NKC, SEQ], BF)
            for c in range(NKC):
                nc.gpsimd.dma_start(
                    out=mt[:, c, :], in_=mT_d[:, c * SEQ:(c + 1) * SEQ]
                )

            # ---- projections ----
            qktmp = const.tile([128, SEQ], BF)   # Q on parts 0:64, K on 64:128
            for g in range(SEQ // 1024):
                qk_ps = stp.tile([128, QT], F32, tag="st")
                for t in range(2):
                    half = slice(t * 512, (t + 1) * 512)
                    for c in range(NCH):
                        nc.tensor.matmul(
                            qk_ps[:, half],
                            wqk[:, c, :], xt[:, 2 * g + t, c, :],
                            start=(c == 0), stop=False,
                        )
                    nc.tensor.matmul(       # +bias: bqk[m] * ones[n]
                        qk_ps[:, half], bqk[0:1, :], ones[0:1, :],
                        start=False, stop=True,
                    )
                nc.vector.tensor_copy(
                    out=qktmp[:, g * 1024:(g + 1) * 1024], in_=qk_ps
                )

            qq = const.tile([128, SEQ], BF)      # Q duplicated on both halves
            kk = const.tile([128, SEQ], BF)      # K duplicated on both halves
            nc.vector.tensor_copy(out=qq[0:64, :], in_=qktmp[0:64, :])
            nc.vector.tensor_copy(out=qq[64:128, :], in_=qktmp[0:64, :])
            nc.vector.tensor_copy(out=kk[0:64, :], in_=qktmp[64:128, :])
            nc.vector.tensor_copy(out=kk[64:128, :], in_=qktmp[64:128, :])

            vp = const.tile([128, NKC, HEAD + 1], BF)   # V' with ones column
            for i in range(4):
                v_ps = scp.tile([128, 4, HEAD], F32, tag="sc")
                for j in range(4):
                    s = 4 * i + j
                    t, jj = s // 4, s % 4
                    for c in range(NCH):
                        nc.tensor.matmul(
                            v_ps[:, j, :],
                            xt[:, t, c, jj * 128:(jj + 1) * 128],
                            wv[:, c, :], start=(c == 0), stop=False,
                        )
                    nc.tensor.matmul(   # +bias: ones[m] * bv[n]
                        v_ps[:, j, :], ones[0:1, 0:128], bv[0:1, :],
                        start=False, stop=True,
                    )
                nc.vector.tensor_copy(
                    out=vp[:, 4 * i:4 * (i + 1), 0:HEAD], in_=v_ps
                )
            nc.vector.memset(vp[:, :, HEAD:HEAD + 1], 1.0)

            # ---- main loop: scores -> exp -> mask -> PV ----
            for q in range(NQT):
                qc = slice(q * QT, (q + 1) * QT)
                ot_ps = otp.tile([HEAD + 1, QT], F32)
                for kp in range(NKC // 2):
                    k0, k1 = 2 * kp, 2 * kp + 1
                    st_a = stp.tile([128, QT], F32, tag="st")
                    st_b = stp.tile([128, QT], F32, tag="st")
                    for h in range(2):
                        qh = slice(q * QT + h * 512, q * QT + (h + 1) * 512)
                        nc.tensor.matmul(
                            st_a[:, h * 512:(h + 1) * 512],
                            kk[0:64, k0 * 128:(k0 + 1) * 128], qq[0:64, qh],
                            start=True, stop=True,
                        )
                        nc.tensor.matmul(
                            st_b[:, h * 512:(h + 1) * 512],
                            kk[64:128, k1 * 128:(k1 + 1) * 128], qq[64:128, qh],
                            start=True, stop=True,
                        )
                    p_a = ppool.tile([128, QT], BF, tag="p")
                    p_b = ppool.tile([128, QT], BF, tag="p")
                    nc.scalar.activation(p_a, st_a, EXP)
                    nc.scalar.activation(p_b, st_b, EXP)
                    nc.vector.tensor_mul(p_a, p_a, mt[:, k0, qc])
                    nc.vector.tensor_mul(p_b, p_b, mt[:, k1, qc])
                    for h in range(2):
                        hs = slice(h * 512, (h + 1) * 512)
                        nc.tensor.matmul(
                            ot_ps[:, hs], vp[:, k0, :], p_a[:, hs],
                            start=(kp == 0), stop=False,
                        )
                        nc.tensor.matmul(
                            ot_ps[:, hs], vp[:, k1, :], p_b[:, hs],
                            start=False, stop=(kp == NKC // 2 - 1),
                        )
                ot_sb = epool.tile([HEAD + 1, QT], F32)
                nc.vector.tensor_copy(out=ot_sb, in_=ot_ps)
                nc.sync.dma_start(out=ot_d[:, qc], in_=ot_sb)

    _split_excess_waits(nc)
    return nc


_CACHE = {}


def _get_nc():
    if "nc" not in _CACHE:
        _CACHE["nc"] = _build()
    return _CACHE["nc"]


def _prep_in_maps(x, Wq, bq, Wk, bk, Wv, bv, mask):
    x = np.asarray(x, dtype=np.float32)
    Wqk = np.concatenate(
        [np.asarray(Wq, np.float32), np.asarray(Wk, np.float32) * 0.125], axis=1
    )
    # partition-major: row p holds [c0 cols | c1 cols | ...] for w = c*128+p
    Wqkh = np.ascontiguousarray(
        Wqk.reshape(NCH, 128, 128).transpose(1, 0, 2).reshape(128, NCH * 128)
    ).astype(BF16)
    Wvh = np.ascontiguousarray(
        np.asarray(Wv, np.float32).reshape(NCH, 128, HEAD)
        .transpose(1, 0, 2).reshape(128, NCH * HEAD)
    ).astype(BF16)
    bqk = np.concatenate(
        [np.asarray(bq, np.float32), np.asarray(bk, np.float32) * 0.125]
    ).astype(BF16).reshape(1, 128)
    bv16 = np.asarray(bv, np.float32).astype(BF16).reshape(1, HEAD)
    # mT[p, c, q] = mask[q, c*128+p], partition-major
    mTh = np.ascontiguousarray(
        np.asarray(mask, np.float32).T.reshape(NKC, 128, SEQ)
        .transpose(1, 0, 2).reshape(128, NKC * SEQ)
    ).astype(BF16)
    in_maps = []
    for b in range(N_CORES):
        # xth[p, t, c, j] = x[b][t*512+j, c*128+p]
        xth = np.ascontiguousarray(
            x[b].reshape(4, 512, NCH, 128).transpose(3, 0, 2, 1)
            .reshape(128, 4 * NCH * 512)
        ).astype(BF16)
        in_maps.append({
            "xT": xth, "Wqk": Wqkh, "Wv": Wvh, "bqk": bqk, "bv": bv16,
            "mT": mTh,
        })
    return in_maps


def _run(in_maps, trace=False, **kw):
    nc = _get_nc()
    return run_bass_kernel_spmd(nc, in_maps, list(range(N_CORES)), trace=trace, **kw)


def kernel(x, Wq, bq, Wk, bk, Wv, bv, mask):
    in_maps = _prep_in_maps(x, Wq, bq, Wk, bk, Wv, bv, mask)
    res = _run(in_maps)
    out = np.empty((N_CORES, SEQ, HEAD), np.float32)
    for b in range(N_CORES):
        ot = np.asarray(res.results[b]["ot"])          # [65, 2048] f32
        out[b] = (ot[:HEAD] / ot[HEAD:HEAD + 1]).T     # normalize + transpose
    return out
